# revision 9
# baseline (speedup 1.0000x reference)
"""MoE (top-2 of 8 experts, GLU-MLP) Trainium2 kernel — expert-parallel over 8 cores.

Strategy:
  - Each core holds one expert's weights (cast to bf16 on load) and the full x.
  - On-device per core: transpose x (PE, fp32), exact fp32 router matmul +
    top-2 sigmoid gating, dense bf16 GLU-MLP for its expert over all tokens,
    gate-scaled partial output.
  - ReduceScatter(add) over the 8 cores combines partial outputs; each core
    returns its token shard, host concatenates.
"""

import numpy as np

import concourse.bass as bass
import concourse.mybir as mybir
import concourse.tile as tile
from concourse import bacc
from concourse.bass_utils import run_bass_kernel_spmd
from concourse.masks import make_identity

FP32 = mybir.dt.float32
BF16 = mybir.dt.bfloat16
P = 128

# problem shapes (hardcoded per contract)
B, S, D, H, E = 4, 2048, 1024, 2048, 8
T = B * S
N_CORES = 8


def build_moe_kernel(T, D, H, E, n_cores, TC=512, use_silu=True):
    """Build the SPMD Bass module. TC = tokens per processing chunk."""
    DC = D // P          # d-chunks of 128
    HC = H // P          # h-chunks of 128
    NT = TC // P         # 128-token tiles per chunk
    NCH = T // TC        # chunks
    TSH = T // n_cores   # output shard rows per core
    ND2 = 2              # d-halves for mm2 output (D/512)
    DH = D // ND2        # 512

    nc = bacc.Bacc("TRN2", target_bir_lowering=False, debug=False,
                   num_devices=n_cores)

    x_d = nc.dram_tensor("x", [T, D], FP32, kind="ExternalInput")
    rw_d = nc.dram_tensor("rw", [E, D], FP32, kind="ExternalInput")
    wg_d = nc.dram_tensor("wg", [D, H], FP32, kind="ExternalInput")
    wu_d = nc.dram_tensor("wu", [D, H], FP32, kind="ExternalInput")
    wd_d = nc.dram_tensor("wd", [H, D], FP32, kind="ExternalInput")
    sel_d = nc.dram_tensor("sel", [P, E], FP32, kind="ExternalInput")
    out_d = nc.dram_tensor("out", [TSH, D], FP32, kind="ExternalOutput")

    with tile.TileContext(nc) as tc:
        with (
            tc.tile_pool(name="wpool", bufs=1) as wpool,
            tc.tile_pool(name="xin", bufs=2) as xin_pool,
            tc.tile_pool(name="xtf", bufs=2) as xtf_pool,
            tc.tile_pool(name="xtb", bufs=2) as xtb_pool,
            tc.tile_pool(name="hp", bufs=1) as h_pool,
            tc.tile_pool(name="sg", bufs=2) as sg_pool,
            tc.tile_pool(name="op", bufs=2) as o_pool,
            tc.tile_pool(name="gp", bufs=2) as g_pool,
            tc.tile_pool(name="ps_tr", bufs=2, space="PSUM") as pstr_pool,
            tc.tile_pool(name="ps_g", bufs=1, space="PSUM") as psg_pool,
            tc.tile_pool(name="ps_u", bufs=1, space="PSUM") as psu_pool,
            tc.tile_pool(name="ps_o", bufs=2, space="PSUM") as pso_pool,
            tc.tile_pool(name="dram", bufs=1, space="DRAM") as dram_pool,
        ):
            # ---- resident tiles ----
            wg_sb = wpool.tile([P, DC, H], BF16)   # [dp, dc, h] = wg[dc*P+dp, h]
            wu_sb = wpool.tile([P, DC, H], BF16)
            wd_sb = wpool.tile([P, HC, D], BF16)   # [hp, hc, d] = wd[hc*P+hp, d]
            rwt_sb = wpool.tile([P, DC, E], FP32)  # [dp, dc, e] = rw[e, dc*P+dp]
            rw_sb = wpool.tile([E, D], FP32)
            sel_sb = wpool.tile([P, E], FP32)
            ident = wpool.tile([P, P], FP32)
            ge_sb = wpool.tile([P, T // P], FP32)  # my-expert gate per token

            make_identity(nc, ident[:])

            # weight loads; gpsimd DMA casts fp32->bf16 inline
            nc.gpsimd.dma_start(
                wg_sb[:], x_ap_rearr(wg_d, "(dc dp) h -> dp dc h", dp=P))
            nc.gpsimd.dma_start(
                wu_sb[:], x_ap_rearr(wu_d, "(dc dp) h -> dp dc h", dp=P))
            nc.gpsimd.dma_start(
                wd_sb[:], x_ap_rearr(wd_d, "(hc hp) d -> hp hc d", hp=P))
            nc.sync.dma_start(rw_sb[:], rw_d.ap())
            nc.sync.dma_start(sel_sb[:], sel_d.ap())

            # transpose router weights on PE: rw [E, D] -> rwT [dp, dc, E]
            rwt_ps = pstr_pool.tile([P, DC, E], FP32, tag="trlg")
            for dc in range(DC):
                nc.tensor.transpose(
                    rwt_ps[:, dc, :], rw_sb[:, dc * P:(dc + 1) * P],
                    ident[:E, :E])
            nc.vector.tensor_copy(rwt_sb[:], rwt_ps[:])

            # DRAM bounce buffers for the collective
            comb_in = dram_pool.tile([T, D], FP32)
            comb_out = dram_pool.tile([TSH, D], FP32)

            for ch in range(NCH):
                t0 = ch * TC
                # -- load x chunk (natural layout, token-tiled) --
                x_nat = xin_pool.tile([P, NT, D], FP32, name="x_nat")
                nc.sync.dma_start(
                    x_nat[:],
                    x_d.ap()[t0:t0 + TC, :].rearrange("(tt p) d -> p tt d", p=P))

                xt_b = xtb_pool.tile([P, DC, TC], BF16, name="xt_b")
                hT = h_pool.tile([P, HC, TC], BF16, name="hT")

                for tt in range(NT):
                    # -- transpose 128 tokens x D (PE), fp32 --
                    ps_tr = pstr_pool.tile([P, DC * P], FP32, tag="trlg")
                    for dc in range(DC):
                        nc.tensor.transpose(
                            ps_tr[:, dc * P:(dc + 1) * P],
                            x_nat[:, tt, dc * P:(dc + 1) * P],
                            ident[:])
                    xt_f = xtf_pool.tile([P, DC, P], FP32, name="xt_f")
                    nc.vector.tensor_copy(
                        xt_f[:].rearrange("p dc t -> p (dc t)"), ps_tr[:])
                    nc.scalar.copy(
                        xt_b[:, :, tt * P:(tt + 1) * P],
                        ps_tr[:].rearrange("p (dc t) -> p dc t", dc=DC))

                    # -- router: logits [t(128), E] fp32, exact --
                    ps_lg = pstr_pool.tile([P, DC * P], FP32, tag="trlg")
                    lg_ps = ps_lg[:, :E]
                    for dc in range(DC):
                        nc.tensor.matmul(
                            lg_ps, lhsT=xt_f[:, dc, :], rhs=rwt_sb[:, dc, :],
                            start=(dc == 0), stop=(dc == DC - 1))

                    # -- top-2 sigmoid gating for my expert --
                    idx = ch * NT + tt
                    lg = g_pool.tile([P, E], FP32, tag="lg")
                    nc.vector.tensor_copy(lg[:], lg_ps)
                    m1 = g_pool.tile([P, 1], FP32, tag="m1")
                    nc.vector.reduce_max(m1[:], lg[:], axis=mybir.AxisListType.X)
                    msk = g_pool.tile([P, E], FP32, tag="msk")
                    nc.vector.tensor_scalar(
                        out=msk[:], in0=lg[:], scalar1=m1[:], scalar2=None,
                        op0=mybir.AluOpType.is_equal)
                    nc.vector.tensor_scalar_mul(msk[:], msk[:], -1e30)
                    nc.vector.tensor_tensor(
                        out=msk[:], in0=lg[:], in1=msk[:],
                        op=mybir.AluOpType.add)
                    m2 = g_pool.tile([P, 1], FP32, tag="m2")
                    nc.vector.reduce_max(m2[:], msk[:], axis=mybir.AxisListType.X)
                    # l_c = <logits, sel>; sel is one-hot for my expert
                    prod = g_pool.tile([P, E], FP32, tag="prod")
                    nc.vector.tensor_tensor(
                        out=prod[:], in0=lg[:], in1=sel_sb[:],
                        op=mybir.AluOpType.mult)
                    lc = g_pool.tile([P, 1], FP32, tag="lc")
                    nc.vector.reduce_sum(lc[:], prod[:], axis=mybir.AxisListType.X)
                    # sigmoids of [m1, m2, lc]
                    sig3 = g_pool.tile([P, 3], FP32, tag="sig3")
                    cat3 = g_pool.tile([P, 3], FP32, tag="cat3")
                    nc.vector.tensor_copy(cat3[:, 0:1], m1[:])
                    nc.vector.tensor_copy(cat3[:, 1:2], m2[:])
                    nc.vector.tensor_copy(cat3[:, 2:3], lc[:])
                    nc.scalar.activation(
                        sig3[:], cat3[:], mybir.ActivationFunctionType.Sigmoid)
                    den = g_pool.tile([P, 1], FP32, tag="den")
                    nc.vector.tensor_tensor(
                        out=den[:], in0=sig3[:, 0:1], in1=sig3[:, 1:2],
                        op=mybir.AluOpType.add)
                    nc.vector.tensor_scalar_add(den[:], den[:], 1e-10)
                    rec = g_pool.tile([P, 1], FP32, tag="rec")
                    nc.vector.reciprocal(rec[:], den[:])
                    keep = g_pool.tile([P, 1], FP32, tag="keep")
                    nc.vector.tensor_tensor(
                        out=keep[:], in0=lc[:], in1=m2[:],
                        op=mybir.AluOpType.is_ge)
                    gtmp = g_pool.tile([P, 1], FP32, tag="gtmp")
                    nc.vector.tensor_tensor(
                        out=gtmp[:], in0=sig3[:, 2:3], in1=rec[:],
                        op=mybir.AluOpType.mult)
                    nc.vector.tensor_tensor(
                        out=ge_sb[:, idx:idx + 1], in0=gtmp[:], in1=keep[:],
                        op=mybir.AluOpType.mult)

                # -- mm1: gate/up projections + SiLU*up -> hT (bf16) --
                for hc in range(HC):
                    ps_g = psg_pool.tile([P, TC], FP32, tag="g")
                    ps_u = psu_pool.tile([P, TC], FP32, tag="u")
                    for dc in range(DC):
                        nc.tensor.matmul(
                            ps_g[:], lhsT=wg_sb[:, dc, hc * P:(hc + 1) * P],
                            rhs=xt_b[:, dc, :],
                            start=(dc == 0), stop=(dc == DC - 1))
                    for dc in range(DC):
                        nc.tensor.matmul(
                            ps_u[:], lhsT=wu_sb[:, dc, hc * P:(hc + 1) * P],
                            rhs=xt_b[:, dc, :],
                            start=(dc == 0), stop=(dc == DC - 1))
                    sgt = sg_pool.tile([P, TC], BF16, tag="sg")
                    if use_silu:
                        nc.scalar.activation(
                            sgt[:], ps_g[:], mybir.ActivationFunctionType.Silu)
                    else:
                        # sim fallback: silu(g) = g * sigmoid(g)
                        nc.scalar.activation(
                            sgt[:], ps_g[:],
                            mybir.ActivationFunctionType.Sigmoid)
                        nc.vector.tensor_tensor(
                            out=sgt[:], in0=sgt[:], in1=ps_g[:],
                            op=mybir.AluOpType.mult)
                    nc.vector.tensor_tensor(
                        out=hT[:, hc, :], in0=sgt[:], in1=ps_u[:],
                        op=mybir.AluOpType.mult)

                # -- mm2: down projection, gate-scale, store --
                for tt in range(NT):
                    idx = ch * NT + tt
                    ot = o_pool.tile([P, D], FP32, name="ot")
                    for dh in range(ND2):
                        ps_o = pso_pool.tile([P, DH], FP32, tag="o")
                        for hc in range(HC):
                            nc.tensor.matmul(
                                ps_o[:], lhsT=hT[:, hc, tt * P:(tt + 1) * P],
                                rhs=wd_sb[:, hc, dh * DH:(dh + 1) * DH],
                                start=(hc == 0), stop=(hc == HC - 1))
                        nc.scalar.activation(
                            ot[:, dh * DH:(dh + 1) * DH], ps_o[:],
                            mybir.ActivationFunctionType.Copy,
                            scale=ge_sb[:, idx:idx + 1])
                    nc.sync.dma_start(
                        comb_in[t0 + tt * P: t0 + (tt + 1) * P, :], ot[:])

            # -- combine across experts: ReduceScatter over token dim --
            nc.gpsimd.collective_compute(
                "ReduceScatter",
                mybir.AluOpType.add,
                ins=[comb_in.opt()],
                outs=[comb_out.opt()],
                replica_groups=[list(range(n_cores))],
            )
            nc.sync.dma_start(out_d.ap(), comb_out[:])

    nc.compile()
    return nc


def x_ap_rearr(dram_tensor, pattern, **kw):
    return dram_tensor.ap().rearrange(pattern, **kw)


_NC_CACHE = {}
_LAST_RES = None


def _get_nc(key, *args, **kw):
    if key not in _NC_CACHE:
        _NC_CACHE[key] = build_moe_kernel(*args, **kw)
    return _NC_CACHE[key]


def kernel(x, router_w, w_gate, w_up, w_down):
    x = np.ascontiguousarray(x, dtype=np.float32)
    router_w = np.ascontiguousarray(router_w, dtype=np.float32)
    w_gate = np.ascontiguousarray(w_gate, dtype=np.float32)
    w_up = np.ascontiguousarray(w_up, dtype=np.float32)
    w_down = np.ascontiguousarray(w_down, dtype=np.float32)

    Bx, Sx, Dx = x.shape
    Tx = Bx * Sx
    Ex, _ = router_w.shape
    Hx = w_gate.shape[2]

    nc = _get_nc(("moe", Tx, Dx, Hx, Ex), Tx, Dx, Hx, Ex, N_CORES)

    xt = x.reshape(Tx, Dx)
    in_maps = []
    for c in range(N_CORES):
        sel = np.zeros((P, Ex), dtype=np.float32)
        sel[:, c] = 1.0
        in_maps.append({
            "x": xt,
            "rw": router_w,
            "wg": np.ascontiguousarray(w_gate[c]),
            "wu": np.ascontiguousarray(w_up[c]),
            "wd": np.ascontiguousarray(w_down[c]),
            "sel": sel,
        })

    res = run_bass_kernel_spmd(nc, in_maps, core_ids=list(range(N_CORES)))
    global _LAST_RES
    _LAST_RES = res
    shards = [res.results[c]["out"] for c in range(N_CORES)]
    out = np.concatenate(shards, axis=0)
    return out.reshape(Bx, Sx, Dx)


if __name__ == "__main__":
    import sys
    sys.path.insert(0, "/root/problem")
    from reference import setup_inputs
    inputs = {k: np.asarray(v) for k, v in setup_inputs().items()}
    out = kernel(**inputs)
    print("kernel output", out.shape, out.dtype)
